# revision 15
# baseline (speedup 1.0000x reference)
"""Trainium2 Bass kernel for the fuzzy joint-membership layer.

Math (derived from the reference 2-qubit circuit, verified vs oracle):
  out[b, 2p,   c] = 0.5 + 0.5*cos(theta_c)*cos(x0) - 0.5*sin(theta_c)*sin(x0)*sin(x1)
  out[b, 2p+1, c] = 0.5 + 0.5*cos(x0)*cos(x1)
where x0 = xf[b, pair_idx[b,p,0]], x1 = xf[b, pair_idx[b,p,1]].

Sharding: pure data parallel, batch 4096 -> 8 cores x 512 rows.

Device kernel per 128-row tile:
  - DMA in xf [128,3072] f32 and indices [128,920] i16
  - gpsimd ap_gather with the natural idx layout: each Q7 core's 16-partition
    group round-robins its 16 rows' index lists, so the useful gathered value
    for partition p (p%16 == s) lands at column 16*j + s of gout [128, 14720]
  - 16 strided SBUF->SBUF DMAs extract the diagonal into gv [128, 920]
  - ACT: cv = sin(gv + pi/2) = cos(vals), sv = sin(vals)
  - DVE: W = sv_even*sv_odd, E = cv_even*cv_odd
  - per class c: out_even_c = (cos(x0)*hct_c + 0.5) + W*(-hst_c)   (2 DVE ops)
                 out_odd_c  = Copy(E*0.5 + 0.5)                    (1 ACT op)
  - DMA out [128, 9200]
"""

import math
import numpy as np

B, PIX, NPAIR, C = 4096, 3072, 460, 10
NG = 2 * NPAIR          # 920 gathered values per row
OUTW = NG * C           # 9200
NCORES = 8
BS = B // NCORES        # 512 rows per core
TILES = BS // 128       # 4

_cache = {}


def _ensure_path():
    try:
        import concourse  # noqa: F401
    except ImportError:
        import sys
        sys.path.insert(0, "/opt/trn_rl_repo")


def build_nc(bs=BS):
    _ensure_path()
    from contextlib import ExitStack
    import concourse.tile as tile
    from concourse import bacc, mybir

    f32, i16 = mybir.dt.float32, mybir.dt.int16
    Sin = mybir.ActivationFunctionType.Sin
    Copy = mybir.ActivationFunctionType.Copy
    mult = mybir.AluOpType.mult
    add = mybir.AluOpType.add
    ntiles = bs // 128

    nc = bacc.Bacc("TRN2", target_bir_lowering=False, debug=False)
    x_ext = nc.declare_dram_parameter("x", [bs, PIX], f32, isOutput=False)
    idx_ext = nc.declare_dram_parameter("idx", [bs, NG], i16, isOutput=False)
    th_ext = nc.declare_dram_parameter("theta", [128, C], f32, isOutput=False)
    mk_ext = nc.declare_dram_parameter("masks", [128, 16 * 128], f32, isOutput=False)
    out_ext = nc.declare_dram_parameter("out", [bs, OUTW], f32, isOutput=True)

    with tile.TileContext(nc) as tc, ExitStack() as ctx:
        cpool = ctx.enter_context(tc.tile_pool(name="const", bufs=1))
        xpool = ctx.enter_context(tc.tile_pool(name="xf", bufs=2))
        ipool = ctx.enter_context(tc.tile_pool(name="idx", bufs=2))
        gpool = ctx.enter_context(tc.tile_pool(name="gout", bufs=2))
        ppool = ctx.enter_context(tc.tile_pool(name="gvp", bufs=2, space="PSUM"))
        tpool = ctx.enter_context(tc.tile_pool(name="trig", bufs=2))
        wpool = ctx.enter_context(tc.tile_pool(name="we", bufs=2))
        opool = ctx.enter_context(tc.tile_pool(name="ot", bufs=2))

        masks = cpool.tile([128, 16 * 128], f32)
        nc.sync.dma_start(out=masks[:], in_=mk_ext[:, :])

        # Scalar-engine Sin only accepts [-pi, pi]. Range-reduce with the
        # round-to-nearest magic trick: n = (v/2pi + M) - M, -r = 2pi*n - v.
        # Then -sin(v) = Sin(-r) and cos(v) = Sin(pi/2 - |r|); the sin sign
        # flip cancels in sin*sin products and is absorbed into nhst.
        sub_ = mybir.AluOpType.subtract
        maxop = mybir.AluOpType.max
        PI, TWO_PI = math.pi, 2 * math.pi
        MAGIC, INV2PI = 1.5 * 2 ** 23, 1.0 / (2 * math.pi)
        pihalf = cpool.tile([128, 1], f32)
        nc.vector.memset(pihalf[:], PI / 2)
        zerob = cpool.tile([128, 1], f32)
        nc.vector.memset(zerob[:], 0.0)

        def trig(pool, src, width, tagp):
            """returns (cv, svN) = (cos(src), -sin(src)), width cols."""
            t1 = pool.tile([128, width], f32, tag=tagp + "t1")
            nc.vector.tensor_scalar(t1[:], src, INV2PI, MAGIC, mult, add)
            nc.vector.tensor_scalar(t1[:], t1[:], MAGIC, None, sub_)
            nc.vector.tensor_scalar(t1[:], t1[:], TWO_PI, None, mult)
            negr = pool.tile([128, width], f32, tag=tagp + "negr")
            nc.vector.tensor_tensor(negr[:], t1[:], src, sub_)
            nc.vector.tensor_scalar(t1[:], negr[:], -1.0, None, mult)
            nc.vector.tensor_tensor(t1[:], t1[:], negr[:], maxop)  # |r|
            cv = pool.tile([128, width], f32, tag=tagp + "cv")
            svN = pool.tile([128, width], f32, tag=tagp + "svN")
            nc.scalar.activation(svN[:], negr[:], Sin, bias=zerob[:, 0:1])
            nc.scalar.activation(cv[:], t1[:], Sin, bias=pihalf[:, 0:1], scale=-1.0)
            return cv, svN

        # theta coefficients: hct = 0.5*cos(theta), nhst = -0.5*sin(theta)
        th_sb = cpool.tile([128, C], f32)
        nc.sync.dma_start(out=th_sb[:], in_=th_ext[:, :])
        cvt, svNt = trig(cpool, th_sb[:], C, "th")
        hcoef = cpool.tile([128, 2 * C], f32)
        nc.vector.tensor_scalar(hcoef[:, 0:C], cvt[:], 0.5, None, mult)
        nc.vector.tensor_scalar(hcoef[:, C:2 * C], svNt[:], 0.5, None, mult)
        hct = hcoef[:, 0:C]        # 0.5*cos(theta)
        nhst = hcoef[:, C:2 * C]   # -0.5*sin(theta) = 0.5*svN

        for t in range(ntiles):
            rows = slice(t * 128, (t + 1) * 128)
            xf = xpool.tile([128, PIX], f32)
            nc.sync.dma_start(out=xf[:], in_=x_ext[rows, :])
            idxt = ipool.tile([128, NG], i16)
            nc.sync.dma_start(out=idxt[:], in_=idx_ext[rows, :])

            # Gather in 4 column-splits so downstream overlaps the serial
            # gpsimd gathers and the Q7 idx scratch stays small. Extract the
            # diagonal (useful value for partition p at col 16j+p%16) via PE:
            # gv = sum_s diag(p%16==s) @ gout[:, s::16], accumulated in PSUM.
            NSPLIT = 10
            HNG = NG // NSPLIT  # 92
            ot = opool.tile([128, OUTW], f32)
            for h in range(NSPLIT):
                gout = gpool.tile([128, 16 * HNG], f32, tag="gout")
                nc.gpsimd.ap_gather(
                    out_ap=gout[:],
                    in_ap=xf[:],
                    idxs_ap=idxt[:, h * HNG:(h + 1) * HNG],
                    channels=128,
                    num_elems=PIX,
                    d=1,
                    num_idxs=16 * HNG,
                )
                gvp = ppool.tile([128, HNG], f32, tag="gvp")
                for s in range(16):
                    rhs = gout[:, s: 16 * HNG: 16]
                    nc.tensor.matmul(
                        gvp[:], masks[:, s * 128:(s + 1) * 128], rhs,
                        start=(s == 0), stop=(s == 15),
                    )
                cv, sv = trig(tpool, gvp[:], HNG, "g")
                w = wpool.tile([128, HNG // 2], f32, tag="w")
                e = wpool.tile([128, HNG // 2], f32, tag="e")
                nc.vector.tensor_tensor(w[:], sv[:, 0:HNG:2], sv[:, 1:HNG:2], mult)
                nc.vector.tensor_tensor(e[:], cv[:, 0:HNG:2], cv[:, 1:HNG:2], mult)

                base = h * (OUTW // NSPLIT)
                for c in range(C):
                    ev = ot[:, base + c: base + OUTW // NSPLIT: 2 * C]
                    nc.vector.tensor_scalar(ev, cv[:, 0:HNG:2], hct[:, c:c + 1], 0.5, mult, add)
                    nc.vector.scalar_tensor_tensor(ev, w[:], nhst[:, c:c + 1], ev, mult, add)
                    ov = ot[:, base + C + c: base + OUTW // NSPLIT: 2 * C]
                    nc.scalar.activation(ov, e[:], Copy, bias=0.5, scale=0.5)
            nc.sync.dma_start(out=out_ext[rows, :], in_=ot[:])

    nc.compile()
    return nc


def _masks_np():
    if "masks" not in _cache:
        m = np.zeros((128, 16 * 128), dtype=np.float32)
        p = np.arange(128)
        m[p, (p % 16) * 128 + p] = 1.0
        _cache["masks"] = np.ascontiguousarray(m)
    return _cache["masks"]


def _get_nc():
    if "nc" not in _cache:
        _cache["nc"] = build_nc()
    return _cache["nc"]


def kernel(x, pair_idx, theta):
    _ensure_path()
    from concourse.bass_utils import run_bass_kernel_spmd

    nc = _get_nc()
    xs = np.ascontiguousarray(np.asarray(x, dtype=np.float32).reshape(B, PIX))
    idx16 = np.ascontiguousarray(
        np.asarray(pair_idx).reshape(B, NG).astype(np.int16)
    )
    thb = np.ascontiguousarray(
        np.tile(np.asarray(theta, dtype=np.float32).reshape(1, C), (128, 1))
    )
    in_maps = [
        {
            "x": xs[k * BS:(k + 1) * BS],
            "idx": idx16[k * BS:(k + 1) * BS],
            "theta": thb,
            "masks": _masks_np(),
        }
        for k in range(NCORES)
    ]
    res = run_bass_kernel_spmd(nc, in_maps, list(range(NCORES))).results
    out = np.concatenate([res[k]["out"] for k in range(NCORES)], axis=0)
    return out.reshape(B, NG, C).astype(np.float32)


# revision 16
# speedup vs baseline: 1.0151x; 1.0151x over previous
"""Trainium2 Bass kernel for the fuzzy joint-membership layer.

Math (derived from the reference 2-qubit circuit, verified vs oracle):
  out[b, 2p,   c] = 0.5 + 0.5*cos(theta_c)*cos(x0) - 0.5*sin(theta_c)*sin(x0)*sin(x1)
  out[b, 2p+1, c] = 0.5 + 0.5*cos(x0)*cos(x1)
where x0 = xf[b, pair_idx[b,p,0]], x1 = xf[b, pair_idx[b,p,1]].

Sharding: pure data parallel, batch 4096 -> 8 cores x 512 rows.

Device kernel per 128-row tile:
  - DMA in xf [128,3072] f32 and indices [128,920] i16
  - gpsimd ap_gather with the natural idx layout: each Q7 core's 16-partition
    group round-robins its 16 rows' index lists, so the useful gathered value
    for partition p (p%16 == s) lands at column 16*j + s of gout [128, 14720]
  - 16 strided SBUF->SBUF DMAs extract the diagonal into gv [128, 920]
  - ACT: cv = sin(gv + pi/2) = cos(vals), sv = sin(vals)
  - DVE: W = sv_even*sv_odd, E = cv_even*cv_odd
  - per class c: out_even_c = (cos(x0)*hct_c + 0.5) + W*(-hst_c)   (2 DVE ops)
                 out_odd_c  = Copy(E*0.5 + 0.5)                    (1 ACT op)
  - DMA out [128, 9200]
"""

import math
import numpy as np

B, PIX, NPAIR, C = 4096, 3072, 460, 10
NG = 2 * NPAIR          # 920 gathered values per row
OUTW = NG * C           # 9200
NCORES = 8
BS = B // NCORES        # 512 rows per core
TILES = BS // 128       # 4

_cache = {}


def _ensure_path():
    try:
        import concourse  # noqa: F401
    except ImportError:
        import sys
        sys.path.insert(0, "/opt/trn_rl_repo")


def build_nc(bs=BS):
    _ensure_path()
    from contextlib import ExitStack
    import concourse.tile as tile
    from concourse import bacc, mybir

    f32, i16 = mybir.dt.float32, mybir.dt.int16
    Sin = mybir.ActivationFunctionType.Sin
    Copy = mybir.ActivationFunctionType.Copy
    mult = mybir.AluOpType.mult
    add = mybir.AluOpType.add
    ntiles = bs // 128

    nc = bacc.Bacc("TRN2", target_bir_lowering=False, debug=False)
    x_ext = nc.declare_dram_parameter("x", [bs, PIX], f32, isOutput=False)
    idx_ext = nc.declare_dram_parameter("idx", [bs, NG], i16, isOutput=False)
    th_ext = nc.declare_dram_parameter("theta", [128, C], f32, isOutput=False)
    mk_ext = nc.declare_dram_parameter("masks", [128, 16 * 128], f32, isOutput=False)
    out_ext = nc.declare_dram_parameter("out", [bs, OUTW], f32, isOutput=True)

    with tile.TileContext(nc) as tc, ExitStack() as ctx:
        cpool = ctx.enter_context(tc.tile_pool(name="const", bufs=1))
        xpool = ctx.enter_context(tc.tile_pool(name="xf", bufs=2))
        ipool = ctx.enter_context(tc.tile_pool(name="idx", bufs=2))
        gpool = ctx.enter_context(tc.tile_pool(name="gout", bufs=2))
        ppool = ctx.enter_context(tc.tile_pool(name="gvp", bufs=2, space="PSUM"))
        tpool = ctx.enter_context(tc.tile_pool(name="trig", bufs=2))
        wpool = ctx.enter_context(tc.tile_pool(name="we", bufs=2))
        opool = ctx.enter_context(tc.tile_pool(name="ot", bufs=2))

        masks = cpool.tile([128, 16 * 128], f32)
        nc.sync.dma_start(out=masks[:], in_=mk_ext[:, :])

        # Scalar-engine Sin only accepts [-pi, pi]. Range-reduce with the
        # round-to-nearest magic trick: n = (v/2pi + M) - M, -r = 2pi*n - v.
        # Then -sin(v) = Sin(-r) and cos(v) = Sin(pi/2 - |r|); the sin sign
        # flip cancels in sin*sin products and is absorbed into nhst.
        sub_ = mybir.AluOpType.subtract
        maxop = mybir.AluOpType.max
        PI, TWO_PI = math.pi, 2 * math.pi
        MAGIC, INV2PI = 1.5 * 2 ** 23, 1.0 / (2 * math.pi)
        pihalf = cpool.tile([128, 1], f32)
        nc.vector.memset(pihalf[:], PI / 2)
        zerob = cpool.tile([128, 1], f32)
        nc.vector.memset(zerob[:], 0.0)

        def trig(pool, src, width, tagp):
            """returns (cv, svN) = (cos(src), -sin(src)), width cols."""
            t1 = pool.tile([128, width], f32, tag=tagp + "t1")
            nc.vector.tensor_scalar(t1[:], src, INV2PI, MAGIC, mult, add)
            nc.vector.tensor_scalar(t1[:], t1[:], MAGIC, None, sub_)
            nc.vector.tensor_scalar(t1[:], t1[:], TWO_PI, None, mult)
            negr = pool.tile([128, width], f32, tag=tagp + "negr")
            nc.vector.tensor_tensor(negr[:], t1[:], src, sub_)
            nc.vector.tensor_scalar(t1[:], negr[:], -1.0, None, mult)
            nc.vector.tensor_tensor(t1[:], t1[:], negr[:], maxop)  # |r|
            cv = pool.tile([128, width], f32, tag=tagp + "cv")
            svN = pool.tile([128, width], f32, tag=tagp + "svN")
            nc.scalar.activation(svN[:], negr[:], Sin, bias=zerob[:, 0:1])
            nc.scalar.activation(cv[:], t1[:], Sin, bias=pihalf[:, 0:1], scale=-1.0)
            return cv, svN

        # theta coefficients: hct = 0.5*cos(theta), nhst = -0.5*sin(theta)
        th_sb = cpool.tile([128, C], f32)
        nc.sync.dma_start(out=th_sb[:], in_=th_ext[:, :])
        cvt, svNt = trig(cpool, th_sb[:], C, "th")
        hcoef = cpool.tile([128, 2 * C], f32)
        nc.vector.tensor_scalar(hcoef[:, 0:C], cvt[:], 0.5, None, mult)
        nc.vector.tensor_scalar(hcoef[:, C:2 * C], svNt[:], 0.5, None, mult)
        hct = hcoef[:, 0:C]        # 0.5*cos(theta)
        nhst = hcoef[:, C:2 * C]   # -0.5*sin(theta) = 0.5*svN

        for t in range(ntiles):
            rows = slice(t * 128, (t + 1) * 128)
            xf = xpool.tile([128, PIX], f32)
            nc.sync.dma_start(out=xf[:], in_=x_ext[rows, :])
            idxt = ipool.tile([128, NG], i16)
            nc.sync.dma_start(out=idxt[:], in_=idx_ext[rows, :])

            # Gather in 4 column-splits so downstream overlaps the serial
            # gpsimd gathers and the Q7 idx scratch stays small. Extract the
            # diagonal (useful value for partition p at col 16j+p%16) via PE:
            # gv = sum_s diag(p%16==s) @ gout[:, s::16], accumulated in PSUM.
            NSPLIT = 4
            HNG = NG // NSPLIT  # 230
            ot = opool.tile([128, OUTW], f32)
            for h in range(NSPLIT):
                gout = gpool.tile([128, 16 * HNG], f32, tag="gout")
                nc.gpsimd.ap_gather(
                    out_ap=gout[:],
                    in_ap=xf[:],
                    idxs_ap=idxt[:, h * HNG:(h + 1) * HNG],
                    channels=128,
                    num_elems=PIX,
                    d=1,
                    num_idxs=16 * HNG,
                )
                gvp = ppool.tile([128, HNG], f32, tag="gvp")
                for s in range(16):
                    rhs = gout[:, s: 16 * HNG: 16]
                    nc.tensor.matmul(
                        gvp[:], masks[:, s * 128:(s + 1) * 128], rhs,
                        start=(s == 0), stop=(s == 15),
                    )
                cv, sv = trig(tpool, gvp[:], HNG, "g")
                w = wpool.tile([128, HNG // 2], f32, tag="w")
                e = wpool.tile([128, HNG // 2], f32, tag="e")
                nc.vector.tensor_tensor(w[:], sv[:, 0:HNG:2], sv[:, 1:HNG:2], mult)
                nc.vector.tensor_tensor(e[:], cv[:, 0:HNG:2], cv[:, 1:HNG:2], mult)

                base = h * (OUTW // NSPLIT)
                for c in range(C):
                    ev = ot[:, base + c: base + OUTW // NSPLIT: 2 * C]
                    nc.vector.tensor_scalar(ev, cv[:, 0:HNG:2], hct[:, c:c + 1], 0.5, mult, add)
                    nc.vector.scalar_tensor_tensor(ev, w[:], nhst[:, c:c + 1], ev, mult, add)
                    ov = ot[:, base + C + c: base + OUTW // NSPLIT: 2 * C]
                    nc.scalar.activation(ov, e[:], Copy, bias=0.5, scale=0.5)
            nc.sync.dma_start(out=out_ext[rows, :], in_=ot[:])

    nc.compile()
    return nc


def _masks_np():
    if "masks" not in _cache:
        m = np.zeros((128, 16 * 128), dtype=np.float32)
        p = np.arange(128)
        m[p, (p % 16) * 128 + p] = 1.0
        _cache["masks"] = np.ascontiguousarray(m)
    return _cache["masks"]


def _get_nc():
    if "nc" not in _cache:
        _cache["nc"] = build_nc()
    return _cache["nc"]


def kernel(x, pair_idx, theta):
    _ensure_path()
    from concourse.bass_utils import run_bass_kernel_spmd

    nc = _get_nc()
    xs = np.ascontiguousarray(np.asarray(x, dtype=np.float32).reshape(B, PIX))
    idx16 = np.ascontiguousarray(
        np.asarray(pair_idx).reshape(B, NG).astype(np.int16)
    )
    thb = np.ascontiguousarray(
        np.tile(np.asarray(theta, dtype=np.float32).reshape(1, C), (128, 1))
    )
    in_maps = [
        {
            "x": xs[k * BS:(k + 1) * BS],
            "idx": idx16[k * BS:(k + 1) * BS],
            "theta": thb,
            "masks": _masks_np(),
        }
        for k in range(NCORES)
    ]
    res = run_bass_kernel_spmd(nc, in_maps, list(range(NCORES))).results
    out = np.concatenate([res[k]["out"] for k in range(NCORES)], axis=0)
    return out.reshape(B, NG, C).astype(np.float32)


# revision 18
# speedup vs baseline: 1.0226x; 1.0073x over previous
"""Trainium2 Bass kernel for the fuzzy joint-membership layer.

Math (derived from the reference 2-qubit circuit, verified vs oracle):
  out[b, 2p,   c] = 0.5 + 0.5*cos(theta_c)*cos(x0) - 0.5*sin(theta_c)*sin(x0)*sin(x1)
  out[b, 2p+1, c] = 0.5 + 0.5*cos(x0)*cos(x1)
where x0 = xf[b, pair_idx[b,p,0]], x1 = xf[b, pair_idx[b,p,1]].

Sharding: pure data parallel, batch 4096 -> 8 cores x 512 rows.

Device kernel per 128-row tile (4 column-split chunks, pipelined):
  - DMA in xf [128,3072] f32 and indices [128,920] i16
  - gpsimd ap_gather with the natural idx layout: each Q7 core's 16-partition
    group round-robins its 16 rows' index lists, so the useful gathered value
    for partition p (p%16 == s) lands at column 16*j + s of gout
    (4 splits of 3680 idx each: smaller calls measure ~7% faster/idx and
    overlap downstream with the serial gpsimd gathers)
  - PE extracts the diagonal: gv = sum_s diag(p%16==s) @ gout[:, s::16]
    accumulated in PSUM (partition-strided SBUF DMA is broken in this stack)
  - DVE range-reduction (magic round) + ACT Sin: cv = cos(vals), sv = -sin(vals)
  - DVE: W = sv_even*sv_odd = sin*sin, E = cv_even*cv_odd
  - per class c: out_even_c = (cos(x0)*hct_c + 0.5) + W*(-hst_c)   (2 DVE ops)
                 out_odd_c  = Copy(E*0.5 + 0.5)                    (1 ACT op)
  - DMA out [128, 9200]
"""

import math
import numpy as np

B, PIX, NPAIR, C = 4096, 3072, 460, 10
NG = 2 * NPAIR          # 920 gathered values per row
OUTW = NG * C           # 9200
NCORES = 8
BS = B // NCORES        # 512 rows per core
TILES = BS // 128       # 4

_cache = {}


def _ensure_path():
    try:
        import concourse  # noqa: F401
    except ImportError:
        import sys
        sys.path.insert(0, "/opt/trn_rl_repo")


def build_nc(bs=BS):
    _ensure_path()
    from contextlib import ExitStack
    import concourse.tile as tile
    from concourse import bacc, mybir

    f32, i16 = mybir.dt.float32, mybir.dt.int16
    Sin = mybir.ActivationFunctionType.Sin
    Copy = mybir.ActivationFunctionType.Copy
    mult = mybir.AluOpType.mult
    add = mybir.AluOpType.add
    ntiles = bs // 128

    nc = bacc.Bacc("TRN2", target_bir_lowering=False, debug=False)
    x_ext = nc.declare_dram_parameter("x", [bs, PIX], f32, isOutput=False)
    idx_ext = nc.declare_dram_parameter("idx", [bs, NG], i16, isOutput=False)
    th_ext = nc.declare_dram_parameter("theta", [128, C], f32, isOutput=False)
    mk_ext = nc.declare_dram_parameter("masks", [128, 16 * 128], f32, isOutput=False)
    out_ext = nc.declare_dram_parameter("out", [bs, OUTW], f32, isOutput=True)

    with tile.TileContext(nc) as tc, ExitStack() as ctx:
        cpool = ctx.enter_context(tc.tile_pool(name="const", bufs=1))
        xpool = ctx.enter_context(tc.tile_pool(name="xf", bufs=2))
        ipool = ctx.enter_context(tc.tile_pool(name="idx", bufs=2))
        gpool = ctx.enter_context(tc.tile_pool(name="gout", bufs=2))
        ppool = ctx.enter_context(tc.tile_pool(name="gvp", bufs=2, space="PSUM"))
        tpool = ctx.enter_context(tc.tile_pool(name="trig", bufs=2))
        wpool = ctx.enter_context(tc.tile_pool(name="we", bufs=2))
        opool = ctx.enter_context(tc.tile_pool(name="ot", bufs=2))

        masks = cpool.tile([128, 16 * 128], f32)
        nc.sync.dma_start(out=masks[:], in_=mk_ext[:, :])

        # Scalar-engine Sin only accepts [-pi, pi]. Range-reduce with the
        # round-to-nearest magic trick: n = (v/2pi + M) - M, -r = 2pi*n - v.
        # Then -sin(v) = Sin(-r) and cos(v) = Sin(pi/2 - |r|); the sin sign
        # flip cancels in sin*sin products and is absorbed into nhst.
        sub_ = mybir.AluOpType.subtract
        maxop = mybir.AluOpType.max
        PI, TWO_PI = math.pi, 2 * math.pi
        MAGIC, INV2PI = 1.5 * 2 ** 23, 1.0 / (2 * math.pi)
        pihalf = cpool.tile([128, 1], f32)
        nc.vector.memset(pihalf[:], PI / 2)
        zerob = cpool.tile([128, 1], f32)
        nc.vector.memset(zerob[:], 0.0)

        def trig(pool, src, width, tagp):
            """returns (cv, svN) = (cos(src), -sin(src)), width cols."""
            t1 = pool.tile([128, width], f32, tag=tagp + "t1")
            nc.vector.tensor_scalar(t1[:], src, INV2PI, MAGIC, mult, add)
            nc.vector.tensor_scalar(t1[:], t1[:], MAGIC, None, sub_)
            nc.vector.tensor_scalar(t1[:], t1[:], TWO_PI, None, mult)
            negr = pool.tile([128, width], f32, tag=tagp + "negr")
            nc.vector.tensor_tensor(negr[:], t1[:], src, sub_)
            nc.vector.tensor_scalar(t1[:], negr[:], -1.0, None, mult)
            nc.vector.tensor_tensor(t1[:], t1[:], negr[:], maxop)  # |r|
            cv = pool.tile([128, width], f32, tag=tagp + "cv")
            svN = pool.tile([128, width], f32, tag=tagp + "svN")
            nc.scalar.activation(svN[:], negr[:], Sin, bias=zerob[:, 0:1])
            nc.scalar.activation(cv[:], t1[:], Sin, bias=pihalf[:, 0:1], scale=-1.0)
            return cv, svN

        # theta coefficients: hct = 0.5*cos(theta), nhst = -0.5*sin(theta)
        th_sb = cpool.tile([128, C], f32)
        nc.sync.dma_start(out=th_sb[:], in_=th_ext[:, :])
        cvt, svNt = trig(cpool, th_sb[:], C, "th")
        hcoef = cpool.tile([128, 2 * C], f32)
        nc.vector.tensor_scalar(hcoef[:, 0:C], cvt[:], 0.5, None, mult)
        nc.vector.tensor_scalar(hcoef[:, C:2 * C], svNt[:], 0.5, None, mult)
        hct = hcoef[:, 0:C]        # 0.5*cos(theta)
        nhst = hcoef[:, C:2 * C]   # -0.5*sin(theta) = 0.5*svN

        for t in range(ntiles):
            rows = slice(t * 128, (t + 1) * 128)
            xf = xpool.tile([128, PIX], f32)
            nc.sync.dma_start(out=xf[:], in_=x_ext[rows, :])
            idxt = ipool.tile([128, NG], i16)
            nc.sync.dma_start(out=idxt[:], in_=idx_ext[rows, :])

            # Gather in 4 column-splits so downstream overlaps the serial
            # gpsimd gathers and the Q7 idx scratch stays small. Extract the
            # diagonal (useful value for partition p at col 16j+p%16) via PE:
            # gv = sum_s diag(p%16==s) @ gout[:, s::16], accumulated in PSUM.
            NSPLIT = 5
            HNG = NG // NSPLIT  # 184
            ot = opool.tile([128, OUTW], f32)
            for h in range(NSPLIT):
                gout = gpool.tile([128, 16 * HNG], f32, tag="gout")
                nc.gpsimd.ap_gather(
                    out_ap=gout[:],
                    in_ap=xf[:],
                    idxs_ap=idxt[:, h * HNG:(h + 1) * HNG],
                    channels=128,
                    num_elems=PIX,
                    d=1,
                    num_idxs=16 * HNG,
                )
                gvp = ppool.tile([128, HNG], f32, tag="gvp")
                for s in range(16):
                    rhs = gout[:, s: 16 * HNG: 16]
                    nc.tensor.matmul(
                        gvp[:], masks[:, s * 128:(s + 1) * 128], rhs,
                        start=(s == 0), stop=(s == 15),
                    )
                cv, sv = trig(tpool, gvp[:], HNG, "g")
                w = wpool.tile([128, HNG // 2], f32, tag="w")
                e = wpool.tile([128, HNG // 2], f32, tag="e")
                nc.vector.tensor_tensor(w[:], sv[:, 0:HNG:2], sv[:, 1:HNG:2], mult)
                nc.vector.tensor_tensor(e[:], cv[:, 0:HNG:2], cv[:, 1:HNG:2], mult)

                base = h * (OUTW // NSPLIT)
                for c in range(C):
                    ev = ot[:, base + c: base + OUTW // NSPLIT: 2 * C]
                    nc.vector.tensor_scalar(ev, cv[:, 0:HNG:2], hct[:, c:c + 1], 0.5, mult, add)
                    nc.vector.scalar_tensor_tensor(ev, w[:], nhst[:, c:c + 1], ev, mult, add)
                    ov = ot[:, base + C + c: base + OUTW // NSPLIT: 2 * C]
                    nc.scalar.activation(ov, e[:], Copy, bias=0.5, scale=0.5)
            nc.sync.dma_start(out=out_ext[rows, :], in_=ot[:])

    nc.compile()
    return nc


def _masks_np():
    if "masks" not in _cache:
        m = np.zeros((128, 16 * 128), dtype=np.float32)
        p = np.arange(128)
        m[p, (p % 16) * 128 + p] = 1.0
        _cache["masks"] = np.ascontiguousarray(m)
    return _cache["masks"]


def _get_nc():
    if "nc" not in _cache:
        _cache["nc"] = build_nc()
    return _cache["nc"]


def kernel(x, pair_idx, theta):
    _ensure_path()
    from concourse.bass_utils import run_bass_kernel_spmd

    nc = _get_nc()
    xs = np.ascontiguousarray(np.asarray(x, dtype=np.float32).reshape(B, PIX))
    idx16 = np.ascontiguousarray(
        np.asarray(pair_idx).reshape(B, NG).astype(np.int16)
    )
    thb = np.ascontiguousarray(
        np.tile(np.asarray(theta, dtype=np.float32).reshape(1, C), (128, 1))
    )
    in_maps = [
        {
            "x": xs[k * BS:(k + 1) * BS],
            "idx": idx16[k * BS:(k + 1) * BS],
            "theta": thb,
            "masks": _masks_np(),
        }
        for k in range(NCORES)
    ]
    res = run_bass_kernel_spmd(nc, in_maps, list(range(NCORES))).results
    out = np.concatenate([res[k]["out"] for k in range(NCORES)], axis=0)
    return out.reshape(B, NG, C).astype(np.float32)
